# revision 48
# baseline (speedup 1.0000x reference)
"""Trainium2 Bass kernel for nn_ArgmaxPositions (argmax-position relevance scatter).

Reference computation (per (i,j,c) of a [39,39,64] grid):
  k* = argmax_{k in 256} patch(i,j)[k] * w[k,c]   (k = (px,py,pc) = px*32+py*4+pc)
  out[4i+px*, 4j+py*, pc*] += rel[i,j,c]
Output: [1,160,160,4] float32.

Distribution (8 NeuronCores, SPMD): shard Cout=64 -> 8 channels per core.
Each core computes argmax+scatter for its channels over the full 39x39 grid
into a private relevance map; ReduceScatter(add) sums the maps and leaves
each core a 20-gx-row slice.

Per-core pipeline (13 tiles of 3 i-rows x 40 j-slots = 120 partitions):
  - channels split: DVE computes prod=patch*w and the per-channel max for
    c3..c7; Pool does the same for c0..c2 (fully disjoint buffers).
  - one-hot * rel, split by channel:
      Act (c3..c7): s = Sign(-prod + mx) in {0,1}; q = Identity(s*(-rel)+rel)
        -> exactly rel at the argmax, 0 elsewhere (per-partition scale/bias APs).
      Pool (c0..c2): per-c fused STT q = (prod == mx_scalar) * rel_bcast.
  - c-reduction: bf16 add tree on DVE -> P[p, 256] bf16.
  - col2im scatter: canvases use a j-SLOT layout (row = 40 slots x 32), so
    writes from adjacent j never superimpose; only i-parity needs 2 canvases.
    3 strided DMAs per tile scatter P straight to DRAM, overlapped.
  - tail: row-aligned readback (row gx = slot*128 + p), pair-add, then one
    shifted add per slot un-slots (each gy sums exactly two (j,py) terms),
    push f32, ReduceScatter(add), copy the 20-row slice out.
"""

import numpy as np

H_IN, W_IN, C_IN = 160, 160, 4
H_OUT, W_OUT, C_OUT = 39, 39, 64
F, S = 8, 4
N_CORES = 8
C_SH = C_OUT // N_CORES          # 8 output channels per core
K = F * F * C_IN                 # 256 patch positions
TILE_I = 3
N_TILES = H_OUT // TILE_I        # 13
JS = 40                          # j-slots per row (39 real + 1 zero pad)
NP = TILE_I * JS                 # 120 partitions per tile
ROW = JS * F * C_IN              # 1280 slot-elements per canvas row
CANV = H_IN * ROW                # 204800 elements per canvas
OUT_FLAT = H_IN * W_IN * C_IN    # 102400
RS_SH = OUT_FLAT // N_CORES      # 12800
GX_SH = H_IN // N_CORES          # 20 output rows per core
WC = W_IN * C_IN                 # 640

# channel assignment: Pool computes one-hot*rel for c in [0, CP), Act the rest;
# DVE computes mult+max for channels [CP, 8), Pool for [0, CP)
CP = 4


def _build_nc(with_tail=True, with_compute=True):
    from contextlib import ExitStack

    from concourse import bass
    import concourse.mybir as mybir

    f32 = mybir.dt.float32
    bf16 = mybir.dt.bfloat16
    AP = bass.AP
    Alu = mybir.AluOpType
    ActF = mybir.ActivationFunctionType

    nc = bass.Bass(target_bir_lowering=False, debug=True)

    patches_ext = nc.declare_dram_parameter(
        "patches", [N_TILES, NP, K], f32, isOutput=False
    )
    w_ext = nc.declare_dram_parameter("w", [C_SH, K], f32, isOutput=False)
    # rel2[p, t, c, 0] = +rel, rel2[p, t, c, 1] = -rel (f32: Act scale/bias APs)
    rel_ext = nc.declare_dram_parameter(
        "rel", [NP, N_TILES, C_SH, 2], f32, isOutput=False
    )
    out_ext = nc.declare_dram_parameter("out", [GX_SH, W_IN, C_IN], f32, isOutput=True)

    canv = nc.dram_tensor("canv", [2, CANV], bf16)     # a = i%2 slot canvases
    dbg_dram = nc.dram_tensor("dbg_dram", [NP * K], bf16)
    dbg_f32 = nc.dram_tensor("dbg_f32", [NP * C_SH], f32)
    ar_in = nc.dram_tensor("ar_in", [OUT_FLAT], f32)
    rs_out = nc.dram_tensor("rs_out", [RS_SH], f32)

    NA = C_SH - CP            # channels on Act
    AI = 2 * NA               # Act instrs per tile

    with ExitStack() as ctx:
        block = ctx.enter_context(nc.Block())
        sem = lambda name: ctx.enter_context(nc.semaphore(name))
        pt_sem = sem("pt_sem")    # patch pair DMAs
        zw_sem = sem("zw_sem")    # w_rep load, Act's channels (c4..7)
        zwb_sem = sem("zwb_sem")  # w_rep load, Pool's channels (c0..3)
        rl_sem = sem("rl_sem")    # rel load
        zc_sem = sem("zc_sem")    # canvas-1 edge zeroing
        z_sem = sem("z_sem")      # zero_t memset + acc guards
        tr_sem = sem("tr_sem")    # DVE progress: max8, 3x STT (+4 per tile)
        pe_sem = sem("pe_sem")    # Pool progress: mult8 (+1 per tile)
        ak_sem = sem("ak_sem")    # Act instr progress (+AI per tile)
        dv_sem = sem("dv_sem")    # DVE tree lvl1/2/3 (+3 per tile)
        fl_sem = sem("fl_sem")    # fill DMAs (+48 per tile)
        rbs_sem = sem("rbs_sem")  # readbacks
        va_sem = sem("va_sem")    # tail adds progress
        ar_sem = sem("ar_sem")    # ar_in pushed / out written
        cc_sem = sem("cc_sem")    # collective done

        sb = lambda *a: ctx.enter_context(nc.sbuf_tensor(*a))
        w_rep = sb("w_rep", [NP, C_SH, K], f32)
        patch_sb = sb("patch_sb", [NP, 4, K], f32)
        rel2 = sb("rel2", [NP, N_TILES, C_SH, 2], f32)
        prod = sb("prod", [NP, 2, C_SH, K], f32)
        mx = sb("mx", [NP, 2, C_SH], f32)
        tmp_s = sb("tmp_s", [NP, K], bf16)          # Act Sign scratch
        q = sb("q", [NP, 2, C_SH, K], bf16)  # channel-major: all writes contiguous
        u = sb("u", [NP, 4, K], bf16)
        v = sb("v", [NP, 2, K], bf16)
        Pbuf = sb("Pbuf", [NP, 3, K], bf16)
        zero_t = sb("zero_t", [128, ROW], bf16)
        # [p, slot, canvas, half-slot h, py%4, pc]; h = j*2 + py//4
        bigrb = sb("bigrb", [128, 2, 2, 2 * JS, 4, C_IN], bf16)
        # 16-elem zero guard in front (h slot 0): h index shifted by 1
        acc2 = sb("acc2", [128, 2, 2 * JS + 1, 4, C_IN], bf16)
        accf = sb("accf", [128, 2, JS, C_IN, C_IN], f32)   # [p, slot, gy//4, gy%4, pc]
        psnap = sb("psnap", [NP, K], bf16)
        psnap2 = sb("psnap2", [NP, K], bf16)

        # ---------------- sync engine: patches + fills + tail DMAs ----------
        @block.sync
        def _(sync: bass.BassEngine):
            if with_compute:
                # patch pairs: tiles (2m, 2m+1) -> slots (2m%4, 2m%4+1)
                def load_pair(m):
                    nt = min(2, N_TILES - 2 * m)
                    sync.dma_start(
                        out=patch_sb[:, (2 * m) % 4 : (2 * m) % 4 + nt, :],
                        in_=AP(
                            patches_ext,
                            2 * m * NP * K,
                            [[K, NP], [NP * K, nt], [1, K]],
                        ),
                    ).then_inc(pt_sem, 16)

                # full w first on one queue: it (plus patch0) gates Pool's
                # first mult and the shared DMA device serializes transfers
                sync.dma_start(
                    out=w_rep[:, :, :],
                    in_=AP(w_ext, 0, [[0, NP], [K, C_SH], [1, K]]),
                ).then_inc(zwb_sem, 16)
                sync.dma_start(
                    out=patch_sb[:, 0:1, :],
                    in_=AP(patches_ext, 0, [[K, NP], [1, K]]),
                ).then_inc(pt_sem, 16)
                sync.dma_start(
                    out=patch_sb[:, 1:2, :],
                    in_=AP(patches_ext, NP * K, [[K, NP], [1, K]]),
                ).then_inc(pt_sem, 16)
                load_pair(1)
                for t in range(N_TILES):
                    # prefetch pair m = t//2 + 2 once tile 2m-3's readers done
                    if t % 2 == 0 and t // 2 + 2 <= (N_TILES - 1) // 2:
                        m = t // 2 + 2
                        sync.wait_ge(pe_sem, 2 * m - 2)
                        load_pair(m)
                    import os as _os3
                    sync.wait_ge(dv_sem, 3 * (t + 1))
                    if with_tail:
                        if t == 0:
                            sync.wait_ge(zc_sem, 16)
                        for il in range(TILE_I):
                            i = TILE_I * t + il
                            a = i % 2
                            sync.dma_start(
                                out=AP(
                                    canv,
                                    a * CANV + 4 * i * ROW,
                                    [[F * C_IN, JS], [ROW, F], [1, F * C_IN]],
                                ),
                                in_=Pbuf[il * JS : (il + 1) * JS, t % 3, :],
                            ).then_inc(fl_sem, 16)
                        if t == 0 and _os3.environ.get("DBG_P0"):
                            sync.dma_start(
                                out=AP(dbg_dram, 0, [[K, NP], [1, K]]),
                                in_=Pbuf[:, 0, :],
                            ).then_inc(fl_sem, 16)
                        if t == 0 and _os3.environ.get("DBG_Q0"):
                            sync.dma_start(
                                out=AP(dbg_f32, 0, [[C_SH, NP], [1, C_SH]]),
                                in_=mx[:, 0, :],
                            ).then_inc(fl_sem, 16)
                            sync.dma_start(
                                out=AP(dbg_dram, 0, [[48, NP], [1, 48]]),
                                in_=qL[:, 0, 0:12, :],
                            ).then_inc(fl_sem, 16)
                            sync.dma_start(
                                out=AP(dbg_dram, NP * 48, [[48, NP], [1, 48]]),
                                in_=qH[:, 0, 0:12, :],
                            ).then_inc(fl_sem, 16)
                        if t == N_TILES - 3:
                            # rows 0-127 (slot 0) final after fills(10)
                            sync.wait_ge(fl_sem, 48 * (N_TILES - 2))
                            sync.dma_start(
                                out=bigrb[:, 0, :, :, :, :],
                                in_=AP(canv, 0, [[ROW, 128], [CANV, 2], [1, ROW]]),
                            ).then_inc(rbs_sem, 16)

            if with_tail:
                # push slot 0 (rows 0-127) once its unslot is done
                sync.wait_ge(va_sem, 2)
                sync.dma_start(
                    out=AP(ar_in, 0, [[WC, 128], [1, WC]]),
                    in_=accf[:, 0, :, :, :],
                ).then_inc(ar_sem, 16)
                sync.wait_ge(va_sem, 4)
                sync.dma_start(
                    out=AP(ar_in, 128 * WC, [[WC, 32], [1, WC]]),
                    in_=accf[0:32, 1, :, :, :],
                ).then_inc(ar_sem, 16)

                sync.wait_ge(cc_sem, 1)
                import os as _os
                if _os.environ.get("DBG_DUMP"):
                    sync.wait_ge(ar_sem, 32)
                else:
                    sync.dma_start(
                        out=AP(out_ext, 0, [[1, RS_SH]]),
                        in_=AP(rs_out, 0, [[1, RS_SH]]),
                    ).then_inc(ar_sem, 16)
                    sync.wait_ge(ar_sem, 48)

        # ---------------- scalar engine (Activation): loads + one-hot -------
        @block.scalar
        def _(scalar: bass.BassScalarEngine):
            scalar.dma_start(
                out=rel2[:, :, :, :], in_=rel_ext[:, :, :, :]
            ).then_inc(rl_sem, 16)
            # preload the activation function table while DMAs stream in
            scalar.activation(
                out=tmp_s[0:1, 0:4],
                in_=tmp_s[0:1, 0:4],
                func=ActF.Sign,
                bias=0.0,
                scale=1.0,
            )
            if with_tail:
                # zero canvas-1 rows {0..3, 156..159} (i odd covers gx 4..155)
                scalar.wait_ge(z_sem, 1)
                scalar.dma_start(
                    out=AP(canv, CANV, [[156 * ROW, 2], [ROW, 4], [1, ROW]]),
                    in_=zero_t[0:8, :],
                ).then_inc(zc_sem, 16)

            if with_compute:
                scalar.wait_ge(rl_sem, 16)
                for t in range(N_TILES):
                    for ci, c in enumerate(range(CP, C_SH)):
                        if ci == 0:
                            # DVE's max lvl2 for tile t done
                            scalar.wait_ge(tr_sem, 5 * t + 1)
                        # s = Sign(-prod + mx) in {0 (argmax), +1}
                        scalar.activation(
                            out=tmp_s[:, :],
                            in_=prod[:, t % 2, c, :],
                            func=ActF.Sign,
                            bias=mx[:, t % 2, c : c + 1],
                            scale=-1.0,
                        ).then_inc(ak_sem, 1)
                        if ci == 0 and t >= 2:
                            # q[t%2] WAR: DVE lvl1(t-2) must have consumed it
                            scalar.wait_ge(dv_sem, 3 * (t - 2) + 1)
                        # q = s*(-rel) + rel  -> rel at argmax, 0 elsewhere
                        scalar.activation(
                            out=q[:, t % 2, c, :],
                            in_=tmp_s[:, :],
                            func=ActF.Identity,
                            bias=rel2[:, t, c, 0:1],
                            scale=rel2[:, t, c, 1:2],
                        ).then_inc(ak_sem, 1)

            if with_tail:
                if with_compute:
                    scalar.wait_ge(fl_sem, 48 * N_TILES)
                else:
                    scalar.wait_ge(zc_sem, 16)
                scalar.dma_start(
                    out=bigrb[0:32, 1, :, :, :, :],
                    in_=AP(canv, 128 * ROW, [[ROW, 32], [CANV, 2], [1, ROW]]),
                ).then_inc(rbs_sem, 16)

        # ---------------- DVE: max8 + STT one-hot c0-2 + add tree -----------
        @block.vector
        def _(vector: bass.BassVectorEngine):
            # canvas-zero source + acc guards: DVE idles at boot anyway
            vector.memset(zero_t[:, :], 0.0)
            vector.memset(acc2[:, :, 0:1, :, :], 0.0)
            vector.memset(acc2[:, 1, :, :, :], 0.0).then_inc(z_sem, 1)

            def tree(tr):
                # q[tr%2] complete: Act(tr) done (own STTs are program-order)
                vector.wait_ge(ak_sem, AI * min(tr + 1, N_TILES - 1))
                vector.tensor_tensor(
                    out=u[:, :, :],
                    in0=q[:, tr % 2, 0:4, :],
                    in1=q[:, tr % 2, 4:8, :],
                    op=Alu.add,
                ).then_inc(dv_sem, 1)
                vector.tensor_tensor(
                    out=v[:, :, :],
                    in0=u[:, 0:2, :],
                    in1=u[:, 2:4, :],
                    op=Alu.add,
                ).then_inc(dv_sem, 1)
                if with_tail and tr >= 3:
                    vector.wait_ge(fl_sem, 48 * (tr - 2))
                vector.tensor_tensor(
                    out=Pbuf[:, tr % 3, :],
                    in0=v[:, 0, :],
                    in1=v[:, 1, :],
                    op=Alu.add,
                ).then_inc(dv_sem, 1)

            if with_compute:
                vector.wait_ge(rl_sem, 16)
                with nc.allow_low_precision("bf16 one-hot relevance pipeline"):
                    for t in range(N_TILES):
                        import os as _os2
                        if _os2.environ.get("DBG_SERIAL") and t >= 2:
                            vector.wait_ge(fl_sem, 48 * (t - 1))
                        # Pool's mult for tile t done
                        vector.wait_ge(pe_sem, t + 1)
                        if t >= 2:
                            # mx[t%2] WAR: Act(t-2) done reading
                            vector.wait_ge(ak_sem, AI * (t - 1))
                        vector.tensor_reduce(
                            out=mx[:, t % 2, :],
                            in_=prod[:, t % 2, :, :],
                            axis=mybir.AxisListType.X,
                            op=Alu.max,
                        ).then_inc(tr_sem, 1)
                        # fused one-hot*rel (all 8 channels on the last
                        # tile so Act retires one tile earlier)
                        for c in range(C_SH if t == N_TILES - 1 else CP):
                            vector.scalar_tensor_tensor(
                                out=q[:, t % 2, c, :],
                                in0=prod[:, t % 2, c, :],
                                scalar=mx[:, t % 2, c : c + 1],
                                in1=rel2[:, t, c, 0]
                                .unsqueeze(1)
                                .to_broadcast([NP, K]),
                                op0=Alu.is_equal,
                                op1=Alu.mult,
                            ).then_inc(tr_sem, 1)
                        if t >= 1:
                            tree(t - 1)
                            import os as _osA
                            if t == 1 and _osA.environ.get("DBG_PSNAP"):
                                vector.tensor_scalar(
                                    out=psnap[:, :],
                                    in0=Pbuf[:, 0, :],
                                    scalar1=1.0,
                                    scalar2=None,
                                    op0=Alu.mult,
                                )
                    tree(N_TILES - 1)
                    import os as _osB
                    if _osB.environ.get("DBG_PSNAP"):
                        vector.tensor_scalar(
                            out=psnap2[:, :],
                            in0=Pbuf[:, (N_TILES - 1) % 3, :],
                            scalar1=1.0,
                            scalar2=None,
                            op0=Alu.mult,
                        )

            if with_tail:
                with nc.allow_low_precision("bf16 canvas sums"):
                    vector.wait_ge(rbs_sem, 16)
                    vector.wait_ge(z_sem, 1)
                    vector.tensor_tensor(
                        out=acc2[:, 0, 1:, :, :],
                        in0=bigrb[:, 0, 0, :, :, :],
                        in1=bigrb[:, 0, 1, :, :, :],
                        op=Alu.add,
                    ).then_inc(va_sem, 1)
                    # un-slot: out[gx, gy, pc] = slot[j(gy), py<4] + slot[j-1, py+4]
                    vector.tensor_tensor(
                        out=accf[:, 0, :, :, :],
                        in0=acc2[:, 0, 1 : 2 * JS + 1 : 2, :, :],
                        in1=acc2[:, 0, 0 : 2 * JS : 2, :, :],
                        op=Alu.add,
                    ).then_inc(va_sem, 1)
                    vector.wait_ge(rbs_sem, 32)
                    vector.tensor_tensor(
                        out=acc2[0:32, 1, 1:, :, :],
                        in0=bigrb[0:32, 1, 0, :, :, :],
                        in1=bigrb[0:32, 1, 1, :, :, :],
                        op=Alu.add,
                    ).then_inc(va_sem, 1)
                    vector.tensor_tensor(
                        out=accf[0:32, 1, :, :, :],
                        in0=acc2[0:32, 1, 1 : 2 * JS + 1 : 2, :, :],
                        in1=acc2[0:32, 1, 0 : 2 * JS : 2, :, :],
                        op=Alu.add,
                    ).then_inc(va_sem, 1)

        # ---------------- Pool: one-hot*rel (c<CP) + lvl1 + collective ------

        @block.gpsimd
        def _(gpsimd: bass.BassGpSimd):
            if with_compute:
                gpsimd.wait_ge(zwb_sem, 16)
                with nc.allow_low_precision("bf16 one-hot relevance pipeline"):
                    for t in range(N_TILES):
                        gpsimd.wait_ge(pt_sem, 16 * (t + 1) if t < 2 else 16 * (t // 2 + 2))
                        if t >= 2:
                            # prod[t%2] WAR: Act(t-2) + DVE(t-2) done reading
                            gpsimd.wait_ge(ak_sem, AI * (t - 1))
                            gpsimd.wait_ge(tr_sem, 5 * (t - 1))
                        gpsimd.tensor_tensor(
                            out=prod[:, t % 2, :, :],
                            in0=patch_sb[:, t % 4, :]
                            .unsqueeze(1)
                            .to_broadcast([NP, C_SH, K]),
                            in1=w_rep[:, :, :],
                            op=Alu.mult,
                        ).then_inc(pe_sem, 1)

            if with_tail:
                import os as _os
                if _os.environ.get("DBG_DUMP"):
                    gpsimd.wait_ge(fl_sem, 48 * N_TILES)
                    _r0 = int(_os.environ.get("DBG_ROW", "0"))
                    _cv = int(_os.environ.get("DBG_CANV", "0"))
                    gpsimd.dma_start(
                        out=AP(out_ext, 0, [[1, 12800]]),
                        in_=AP(canv, _cv * CANV + _r0 * ROW, [[1, 12800]]),
                    ).then_inc(ar_sem, 16)
                gpsimd.wait_ge(ar_sem, 32)
                gpsimd.collective_compute(
                    "ReduceScatter",
                    mybir.AluOpType.add,
                    replica_groups=[list(range(N_CORES))],
                    ins=[ar_in[:]],
                    outs=[rs_out[:]],
                ).then_inc(cc_sem, 1)

    return nc


_NC = None


def _get_nc():
    global _NC
    if _NC is None:
        _NC = _build_nc()
    return _NC


LAST_RESULT = None


def kernel(inputs, layer_output, layer_weights, stride=4, filter_size=8, **_kw):
    assert int(stride) == S and int(filter_size) == F
    rel = np.asarray(inputs, dtype=np.float32)[0]          # [39,39,64]
    x = np.ascontiguousarray(np.asarray(layer_output, dtype=np.float32)[0])
    w = np.asarray(layer_weights, dtype=np.float32)        # [8,8,4,64]

    import ml_dtypes

    # host-side im2col in (t, il*40+j, k) layout, natural j order, j=39 padded
    idx_r = (S * np.arange(H_OUT))[:, None] + np.arange(F)[None, :]
    idx_c = (S * np.arange(W_OUT))[:, None] + np.arange(F)[None, :]
    pat = x[idx_r][:, :, idx_c, :]                    # [i, px, j, py, pc]
    pat = pat.transpose(0, 2, 1, 3, 4).reshape(H_OUT, W_OUT, K)
    pat40 = np.concatenate([pat, pat[:, -1:, :]], axis=1)   # pad j=39 (finite)
    patches = np.ascontiguousarray(pat40.reshape(N_TILES, NP, K))

    from concourse.bass_utils import run_bass_kernel_spmd

    nc = _get_nc()
    in_maps = []
    for r in range(N_CORES):
        cs = slice(C_SH * r, C_SH * (r + 1))
        w_t = np.ascontiguousarray(
            w[:, :, :, cs].transpose(3, 0, 1, 2).reshape(C_SH, K)
        )
        rel_r = rel[:, :, cs]                              # [39, 39, 8]
        rel_p = np.zeros((H_OUT, JS, C_SH, 2), dtype=np.float32)
        rel_p[:, :W_OUT, :, 0] = rel_r
        rel_p[:, :W_OUT, :, 1] = -rel_r
        rel_p = np.ascontiguousarray(
            rel_p.reshape(N_TILES, NP, C_SH, 2).transpose(1, 0, 2, 3)
        )
        in_maps.append({"patches": patches, "w": w_t, "rel": rel_p})

    import os

    trace = bool(int(os.environ.get("KERNEL_TRACE", "0")))
    res = run_bass_kernel_spmd(nc, in_maps, list(range(N_CORES)), trace=trace)
    global LAST_RESULT
    LAST_RESULT = res
    slices = [np.asarray(res.results[r]["out"]) for r in range(N_CORES)]
    out = np.concatenate(slices, axis=0).reshape(1, H_IN, W_IN, C_IN)
    return out.astype(np.float32)
